# revision 29
# baseline (speedup 1.0000x reference)
"""Multi-head attention (N=2048, D=1024, H=16) on 8 TRN2 NeuronCores.

Sharding: tensor-parallel over heads (2 heads / core). x is replicated
(pre-transposed + pre-cast on host), each core computes QKV / scores /
softmax / PV / out-proj for its 2 heads, producing a partial (N, D)
projection output in fp16. The all-reduce over cores is the host-side
f64 sum of the 8 partials (+ b_proj), cast back to f32.

Device inputs (per core):
  xT      (D, N)    bf16 : x transposed (host prep)
  wqkvT   (128,3,8,128) bf16 : [Wq.T | Wk.T | Wv.T] slices, part-major
  wpT     (128, D)  bf16 : w_proj[:, core_cols].T
  bqkv    (128, 3)  f32  : [bq | bk | bv] slices
  out_part(N, D)    f16  : partial projection output

Per-core pipeline (ACT-bound design; exp on the scalar engine is the
64 x 1024-elem roofline, every other engine hides under it):
  QKV   bf16 matmuls j-sliced; x streams per j-slice (j0 split in two
        o-halves riding a reordered DMA queue), K(j) is always the next
        x-slice off the wire so block-0 scores are never queued behind
        lower-priority PE work. V is computed seq-major directly
        (lhsT = x chunk, rhs = wv chunk) into [V_h0|1|V_h1|1]; its bias
        rides a K=1 ones-row matmul (exact through the softmax mean).
  scores lhsT = K.T slice, rhs = Q.T (bf16; fp8e4-DoubleRow knob exists
        but costs ~2.4e-2 rel err - over the gate).
  exp   one ACT pass per m-chunk into per-block es arrays (bufs=4).
  PV    deferred one-or-more blocks behind exp via a work queue popped
        into later blocks' PE slack: per (qc, h) stream, lhsT = es chunk
        (128 keys x 128 q), rhs = [V|1] (128 keys x 65, ap_size 65).
        PSUM accumulators are pre-memset and all matmuls use
        start=False (PSUM "start" zeroes whole banks on this target),
        so streams share banks without clobbering and without WAR
        chains. The last block accumulates mc-major, riding along its
        own exp stream so only division remains after the last exp.
  div   rowsum is column 64 (q-major): DVE reciprocal + per-partition
        broadcast mul -> O bf16. PE transpose -> O.T.
  proj  O.T^T @ wpT in bf16 -> fp16 partial, one DMA per 128x512 chunk.
        The last block's division/projection chains alternate DVE/ACT
        and are emitted breadth-first to pipeline the tail.
"""

import os
import sys

import numpy as np

for _p in ("/opt/trn_rl_repo",):
    if os.path.isdir(_p) and _p not in sys.path:
        sys.path.insert(0, _p)

N, D, H = 2048, 1024, 16
DH = D // H                 # 64
NCORES = 8
HPC = H // NCORES           # 2 heads per core
P = 128
SCALE = 1.0 / DH ** 0.5

D_CHUNKS = D // P           # 8

# config knobs (overridable before run() for experiments)
NB = int(os.environ.get("ATTN_NB", "512"))                 # query-block size
WARMUP = int(os.environ.get("ATTN_WARMUP", "24"))          # PE warmup transposes
ES_BUFS = int(os.environ.get("ATTN_ES_BUFS", "4"))         # es block arrays

# fp8 DoubleRow scores: j-slices (4 key m-chunks each) whose K is stored as
# fp8 (hi, residual) members; scores for those key chunks run the PE at 2x.
# K-side quantization is exactly compensated by the residual member; only the
# Q-side fp8 rounding contributes error (~1.6e-2 rel-absmax at 4/4 coverage).
FP8_JS = frozenset(
    int(c) for c in os.environ.get("ATTN_FP8_JS", "0123") if c.isdigit()
)
# (b*16+mc) exp tiles computed on DVE via bf16-Schraudolph instead of ACT
DVE_EXP = frozenset(
    int(s) for s in os.environ.get("ATTN_DVE_EXP", "").split(",") if s.strip()
)
LOG2E = 1.4426950408889634
EXP_A = 128.0 * LOG2E * SCALE        # folds softmax scale into the bitcast exp
EXP_B = 16256.0 - float(os.environ.get("ATTN_EXP_C", "5.600"))


def _build_nc(n=N, nb=NB):
    """Build the per-core Bass module (SPMD: identical program, per-core data)."""
    import concourse.bass as bass  # noqa: F401
    import concourse.mybir as mybir
    import concourse.tile as tile
    from concourse import bacc
    from concourse.masks import make_identity

    f32 = mybir.dt.float32
    bf16 = mybir.dt.bfloat16
    f16 = mybir.dt.float16
    fp8 = mybir.dt.float8e4
    i16 = mybir.dt.int16
    AF = mybir.ActivationFunctionType
    DR = mybir.MatmulPerfMode.DoubleRow
    ALU = mybir.AluOpType

    m_chunks = n // P           # 16
    n_blocks = n // nb          # 4
    QC = nb // P                # 4 query chunks per block

    nc = bacc.Bacc(
        "TRN2",
        target_bir_lowering=False,
        debug=False,
        enable_asserts=True,
        num_devices=NCORES,
    )

    xT_d = nc.dram_tensor("xT", (D, n), bf16, kind="ExternalInput")
    wqkvT_d = nc.dram_tensor("wqkvT", (P, 3, D_CHUNKS, P), bf16, kind="ExternalInput")
    wpT_d = nc.dram_tensor("wpT", (P, D), bf16, kind="ExternalInput")
    bqkv_d = nc.dram_tensor("bqkv", (P, 3), f32, kind="ExternalInput")
    bvrow_d = nc.dram_tensor("bvrow", (1, P), bf16, kind="ExternalInput")
    out_d = nc.dram_tensor("out_part", (n, D), f16, kind="ExternalOutput")

    with tile.TileContext(nc) as tc:
        with (
            tc.tile_pool(name="consts", bufs=1) as consts,
            tc.tile_pool(name="xpool", bufs=1) as xpool,
            tc.tile_pool(name="qkpool", bufs=1) as qkpool,
        ):
            # ---- inputs ----
            wqkv_sb = consts.tile([P, 3, D_CHUNKS, P], bf16)
            wp_sb = consts.tile([P, D], bf16)
            bqkv_sb = consts.tile([P, 3], f32)
            xT_sb = xpool.tile([P, D_CHUNKS, n], bf16)

            qw = min(512, n)
            n_j = n // qw
            # one DMA per j-slice of x, except j0 which is split in two
            # o-halves so q/k start accumulating while half 2 streams in;
            # order: x-h1, wq, wk, x-h2 puts the startup chain on the
            # critical path exactly once
            xT_re = xT_d.ap().rearrange("(o p) x -> p o x", p=P)

            def x_slice(j, o0=0, o1=D_CHUNKS):
                nc.sync.dma_start(
                    xT_sb[:, o0:o1, j * qw:(j + 1) * qw],
                    xT_re[:, o0:o1, j * qw:(j + 1) * qw],
                )

            # whole-part weight DMAs keep 2KB contiguous runs per partition
            # (o-sliced weight DMAs drop to 256B runs = 2x descriptor cost);
            # x rides in halves so q's first matmuls start off half 1
            x_slice(0, 0, 4)
            nc.sync.dma_start(wqkv_sb[:, 0], wqkvT_d.ap()[:, 0])
            x_slice(0, 4, 8)
            nc.sync.dma_start(wqkv_sb[:, 1], wqkvT_d.ap()[:, 1])
            nc.sync.dma_start(bqkv_sb[:], bqkv_d.ap())
            nc.sync.dma_start(wqkv_sb[:, 2], wqkvT_d.ap()[:, 2])
            for j in range(1, n_j):
                x_slice(j, 0, 4)
                x_slice(j, 4, 8)
            nc.sync.dma_start(wp_sb[:], wpT_d.ap())

            ident = consts.tile([P, P], bf16)
            make_identity(nc, ident[:])

            # ---- persistent activations ----
            # fp8 DoubleRow members: kT8 = (fp8(K), K - fp8(K)); qT8 = fp8(Q)
            # broadcast across the member axis at the matmul, so the DR
            # contraction computes (fp8(K) + Klo) . fp8(Q) = K . fp8(Q).
            any_fp8 = bool(FP8_JS)
            any_bf = len(FP8_JS) < n // min(512, n)
            if any_fp8:
                qT8_sb = qkpool.tile([P, 1, n], fp8)
                kT8_sb = qkpool.tile([P, 2, n], fp8)
            if any_bf:
                qT_sb = qkpool.tile([P, 1, n], bf16)
                kT_sb = qkpool.tile([P, 1, n], bf16)
            # [V_h0|1|V_h1|1] seq-major, per m-chunk
            v_sb = qkpool.tile([P, m_chunks, 2 * (DH + 1)], bf16)
            nc.gpsimd.memset(v_sb[:, :, DH:DH + 1], 1.0)
            nc.gpsimd.memset(v_sb[:, :, 2 * DH + 1:2 * DH + 2], 1.0)
            # K=1 ones row + bv row: folds the V bias into the PV average
            # (sum_k p_k (v+bv) = O + bv exactly, rowsum column unaffected)
            ones_row = consts.tile([P, P], bf16)
            nc.gpsimd.memset(ones_row[0:1, :], 1.0)
            bvrow_sb = consts.tile([P, P], bf16)
            nc.sync.dma_start(bvrow_sb[0:1, :], bvrow_d.ap())
            # es arrays: one full block of exp(S) per buffer; 3 bufs so the
            # exp stream of block b never waits on PV of block b-ES_BUFS+1
            es_arr = [
                qkpool.tile([P, m_chunks, 2 * nb], bf16, name=f"es_arr{i}")
                for i in range(ES_BUFS)
            ]

            # ===== PSUM pools (8 banks): sps 2x2 + pvps 2x1 + accp 2x1 =====
            accp = tc.alloc_tile_pool(name="accp", bufs=2, space="PSUM")
            sps = tc.alloc_tile_pool(name="sps", bufs=2, space="PSUM")
            pvps = tc.alloc_tile_pool(name="pvps", bufs=2, space="PSUM")

            for wi in range(WARMUP):
                wt = accp.tile([P, P], bf16, tag="acc", name=f"warm_{wi}")
                nc.tensor.transpose(wt[:], ident[:], ident[:])

            def drain_q(j, ps, c0, c1):
                sl = slice(j * qw + c0, j * qw + c1)
                bias = bqkv_sb[:, 0:1].broadcast_to([P, c1 - c0])
                if any_fp8:
                    nc.vector.tensor_add(qT8_sb[:, 0, sl], ps[:, c0:c1], bias)
                if any_bf:
                    nc.vector.tensor_add(qT_sb[:, 0, sl], ps[:, c0:c1], bias)

            def drain_k(j, ps, c0, c1):
                sl = slice(j * qw + c0, j * qw + c1)
                bias = bqkv_sb[:, 1:2].broadcast_to([P, c1 - c0])
                if j in FP8_JS:
                    # member0 = fp8(K); member1 = K - fp8(K) (exact residual)
                    nc.vector.tensor_add(kT8_sb[:, 0, sl], ps[:, c0:c1], bias)
                    nc.vector.scalar_tensor_tensor(
                        kT8_sb[:, 1, sl], ps[:, c0:c1], bqkv_sb[:, 1:2],
                        kT8_sb[:, 0, sl], op0=ALU.add, op1=ALU.subtract,
                    )
                else:
                    nc.vector.tensor_add(kT_sb[:, 0, sl], ps[:, c0:c1], bias)

            SKIP_PARTS = set(
                int(c) for c in os.environ.get("ATTN_SKIP_PARTS", "")
                if c.isdigit()
            )

            def qkv_j(j, parts):
                for part in parts:
                    if part in SKIP_PARTS:
                        continue
                    if part == 2:
                        # V computed seq-major directly: lhsT = x chunk
                        # (stationary), rhs = wv chunk -> out [128 seq, 128
                        # dh]; the K=1 ones-row matmul folds in the V bias.
                        # One PSUM bank holds all 4 m-chunks of the j-slice:
                        # pre-memset + start=False accumulation everywhere
                        # (PSUM "start" zeroes whole banks on this target).
                        nmc = qw // P
                        mc0 = j * qw // P
                        vp = accp.tile([P, nmc, P], f32, tag="acc",
                                       name=f"v_ps_{j}")
                        nc.vector.memset(vp[:], 0.0)
                        for i in range(nmc):
                            msl = slice((mc0 + i) * P, (mc0 + i + 1) * P)
                            for o in range(D_CHUNKS):
                                nc.tensor.matmul(
                                    vp[:, i, :],
                                    xT_sb[:, o, msl],
                                    wqkv_sb[:, 2, o, :],
                                    start=False, stop=False,
                                    skip_group_check=True,
                                )
                            nc.tensor.matmul(
                                vp[:, i, :], ones_row[0:1, :], bvrow_sb[0:1, :],
                                start=False, stop=(i == nmc - 1),
                                skip_group_check=True,
                            )
                        # one strided copy for both heads: dst views the
                        # [v0|1|v1|1] layout as [mc, head, DH] (strides
                        # 2*(DH+1), DH+1, 1), skipping the ones columns
                        dst = v_sb[:, mc0:mc0 + nmc, :].rearrange(
                            "p m (h d) -> p m h d", h=2, d=DH + 1
                        )[:, :, :, 0:DH]
                        nc.vector.tensor_copy(
                            dst, vp[:, :, :].rearrange("p m (h d) -> p m h d",
                                                       h=2, d=DH)
                        )
                        continue
                    ps = accp.tile([P, qw], f32, tag="acc",
                                   name=f"qkv_ps_{part}_{j}")
                    for o in range(D_CHUNKS):
                        nc.tensor.matmul(
                            ps[:], wqkv_sb[:, part, o, :],
                            xT_sb[:, o, j * qw:(j + 1) * qw],
                            start=(o == 0), stop=(o == D_CHUNKS - 1),
                        )
                    with nc.allow_low_precision(reason="qk fp8 drain"):
                        if part == 1:
                            # split the K drain so the slice's first m-chunks
                            # unblock their scores before the full drain
                            for c0, c1 in ((0, P), (P, qw)):
                                drain_k(j, ps, c0, c1)
                        else:
                            drain_q(j, ps, 0, qw)

            def qkv_j0_qk(chunk_cb=None):
                """q+k for j0 over the split x DMA. k is computed per
                128-key column chunk (pre-memset PSUM + start=False shares
                the bank without re-zeroing) and drained chunk-by-chunk, so
                chunk_cb(c) can emit that chunk's scores immediately — the
                first exp starts ~3 m-chunks of matmul earlier."""
                pss = {}
                for part in (0, 1):
                    pss[part] = accp.tile([P, qw], f32, tag="acc",
                                          name=f"qkv_ps_{part}_0")
                nc.vector.memset(pss[1][:], 0.0)
                if 0 not in SKIP_PARTS:
                    for o in range(D_CHUNKS):
                        nc.tensor.matmul(
                            pss[0][:], wqkv_sb[:, 0, o, :],
                            xT_sb[:, o, 0:qw],
                            start=(o == 0), stop=(o == D_CHUNKS - 1),
                        )
                    with nc.allow_low_precision(reason="qk fp8 drain"):
                        drain_q(0, pss[0], 0, qw)
                if 1 in SKIP_PARTS:
                    return
                if os.environ.get("ATTN_J0_CHUNK", "1") == "0":
                    for o in range(D_CHUNKS):
                        nc.tensor.matmul(
                            pss[1][:], wqkv_sb[:, 1, o, :], xT_sb[:, o, 0:qw],
                            start=False, stop=(o == D_CHUNKS - 1),
                            skip_group_check=True,
                        )
                    with nc.allow_low_precision(reason="qk fp8 drain"):
                        drain_k(0, pss[1], 0, P)
                        drain_k(0, pss[1], P, qw)
                    for c in range(qw // P):
                        if chunk_cb is not None:
                            chunk_cb(c)
                    return
                for c in range(qw // P):
                    for o in range(D_CHUNKS):
                        nc.tensor.matmul(
                            pss[1][:, c * P:(c + 1) * P],
                            wqkv_sb[:, 1, o, :],
                            xT_sb[:, o, c * P:(c + 1) * P],
                            start=False, stop=(o == D_CHUNKS - 1),
                            skip_group_check=True,
                        )
                    with nc.allow_low_precision(reason="qk fp8 drain"):
                        drain_k(0, pss[1], c * P, (c + 1) * P)
                    if chunk_cb is not None:
                        chunk_cb(c)

            with (
                tc.tile_pool(name="opool", bufs=4) as opool,
                tc.tile_pool(name="otpool", bufs=4) as otpool,
                tc.tile_pool(name="outpool", bufs=5) as outpool,
                tc.tile_pool(name="zrpool", bufs=10) as zrpool,
            ):
                TRUNC = int(os.environ.get("ATTN_TRUNC", "0"))
                exp_count = [0]

                def scores_exp(b, row0, nbb, mc):
                    """Scores + exp for one m-chunk -> es_arr[b%ES][:, mc]."""
                    if TRUNC and exp_count[0] >= TRUNC:
                        return
                    exp_count[0] += 1
                    nsl = slice(row0, row0 + nbb)
                    on_dve = b * m_chunks + mc in DVE_EXP
                    s_ps = sps.tile([P, 2 * nbb], f32, tag="s",
                                    name=f"s_ps_{b}_{mc}")
                    fp8j = (mc * P // qw) in FP8_JS
                    for h in range(HPC):
                        hsl = slice(h * DH, (h + 1) * DH)
                        st = sp = True
                        if fp8j:
                            nc.tensor.matmul(
                                s_ps[:, h * nbb:(h + 1) * nbb],
                                kT8_sb[hsl, :, mc * P:(mc + 1) * P],
                                qT8_sb[hsl, :, nsl].broadcast_to([DH, 2, nbb]),
                                perf_mode=DR,
                                start=st, stop=sp, skip_group_check=True,
                            )
                        else:
                            nc.tensor.matmul(
                                s_ps[:, h * nbb:(h + 1) * nbb],
                                kT_sb[hsl, 0, mc * P:(mc + 1) * P],
                                qT_sb[hsl, 0, nsl],
                                start=st, stop=sp, skip_group_check=True,
                            )
                    es_out = es_arr[b % ES_BUFS][:, mc, :]
                    if on_dve:
                        # bf16 Schraudolph: exp(SCALE*s) ~ bitcast16(int16(
                        # s*EXP_A + EXP_B)); one DVE op straight from PSUM
                        with nc.allow_low_precision(reason="schraudolph exp"):
                            nc.vector.tensor_scalar(
                                es_out.bitcast(i16), s_ps[:], EXP_A, EXP_B,
                                op0=ALU.mult, op1=ALU.add,
                            )
                    else:
                        nc.scalar.activation(es_out, s_ps[:], AF.Exp,
                                             scale=SCALE)

                # PV PSUM: one tile holds both heads of one q-chunk pair;
                # pre-memset + start=False so streams never clobber (PSUM
                # "start" zeroes whole banks on this target) and no
                # stream-to-stream WAR chaining through bank reuse.
                pv_tiles = {}

                def pv_qcpair_psum(b, qp):
                    pv = pvps.tile([P, 2 * HPC, DH + 1], f32, tag="pv",
                                   name=f"pvp_{b}_{qp}")
                    nc.vector.memset(pv[:], 0.0)
                    return pv

                def pv_stream(b, nbb, qc, h):
                    """One PV accumulation stream into its qc-pair tile."""
                    es = es_arr[b % ES_BUFS]
                    key = (b, qc // 2)
                    if key not in pv_tiles:
                        pv_tiles[key] = pv_qcpair_psum(b, qc // 2)
                    pv = pv_tiles[key][:, (qc % 2) * HPC + h, :]
                    for mc in range(m_chunks):
                        nc.tensor.matmul(
                            pv,
                            es[:, mc, h * nbb + qc * P:h * nbb + (qc + 1) * P],
                            v_sb[:, mc, h * (DH + 1):(h + 1) * (DH + 1)],
                            start=False,
                            stop=(mc == m_chunks - 1),
                            skip_group_check=True,
                        )
                    return pv

                def division(b, qc, h, pv, ov, on_act=False):
                    """O[:, h] = O'/rowsum (q-major). The reciprocal is DVE;
                    the mul can run on ACT (scaled copy, per-partition zr)
                    when DVE is the tail bottleneck."""
                    zr = zrpool.tile([P, 1], f32, tag="zr",
                                     name=f"zr_{b}_{qc}_{h}")
                    with nc.allow_low_precision(reason="softmax recip"):
                        nc.vector.reciprocal(zr[:], pv[:, DH:DH + 1])
                    if on_act:
                        nc.scalar.mul(ov[:, h, :], pv[:, 0:DH], zr[:])
                    else:
                        nc.vector.tensor_mul(
                            ov[:, h, :], pv[:, 0:DH],
                            zr[:].broadcast_to([P, DH]),
                        )

                def pv_div_qc(b, nbb, qc):
                    """Both heads' PV streams + divisions for one q-chunk."""
                    ov = opool.tile([P, HPC, DH], bf16, tag="o",
                                    name=f"o_{b}_{qc}")
                    for h in range(HPC):
                        pv = pv_stream(b, nbb, qc, h)
                        division(b, qc, h, pv, ov)
                    return ov

                def pv_mc_last(b, nbb, qp, mcs):
                    """mc-major PV for the last block: accumulate the given
                    m-chunks for all 4 streams of one qc-pair."""
                    es = es_arr[b % ES_BUFS]
                    key = (b, qp)
                    if key not in pv_tiles:
                        pv_tiles[key] = pv_qcpair_psum(b, qp)
                    pv = pv_tiles[key]
                    for mc in mcs:
                        for qi in range(2):
                            qc = qp * 2 + qi
                            for h in range(HPC):
                                nc.tensor.matmul(
                                    pv[:, qi * HPC + h, :],
                                    es[:, mc,
                                       h * nbb + qc * P:h * nbb + (qc + 1) * P],
                                    v_sb[:, mc,
                                         h * (DH + 1):(h + 1) * (DH + 1)],
                                    start=False,
                                    stop=(mc == m_chunks - 1),
                                    skip_group_check=True,
                                )

                def div_proj_last(b, row0, nbb):
                    """Tail: divisions + projections for the last block.

                    Emitted breadth-first (all recips, all muls, ...) with
                    qc-chains alternating DVE/ACT, so each engine streams
                    same-type items back-to-back and the four chains
                    pipeline instead of serializing."""
                    nqc = nbb // P
                    ovs, oTps, oTs = {}, {}, {}
                    # one reciprocal + one broadcast-mul per qc-PAIR tile
                    # (strided over the 4 stream rowsums) halves the number
                    # of cross-engine hops in the tail
                    for qp in range(nqc // 2):
                        pv = pv_tiles[(b, qp)]
                        zr = zrpool.tile([P, 2 * HPC, 1], f32, tag="zr4",
                                         name=f"zr_{b}_{qp}")
                        with nc.allow_low_precision(reason="softmax recip"):
                            nc.vector.reciprocal(zr[:], pv[:, :, DH:DH + 1])
                        ov = opool.tile([P, 2 * HPC, DH], bf16, tag="o",
                                        name=f"o_{b}_{qp}")
                        nc.vector.tensor_mul(
                            ov[:], pv[:, :, 0:DH],
                            zr[:].broadcast_to([P, 2 * HPC, DH]),
                        )
                        ovs[qp] = ov
                    # leftover queued projection rides here: after the tail
                    # divisions grabbed the DVE, before the PE transposes
                    while projq:
                        batch = [projq.popleft() for _ in range(min(2, len(projq)))]
                        project_stage(batch)
                    for qc in range(nqc):
                        oTps[qc] = accp.tile([P, P], bf16, tag="acc",
                                             name=f"oT_ps_{b}_{qc}")
                        nc.tensor.transpose(
                            oTps[qc][:],
                            ovs[qc // 2][:, (qc % 2) * HPC:(qc % 2 + 1) * HPC, :],
                            ident[:],
                        )
                        oTs[qc] = otpool.tile([P, P], bf16, tag="oT",
                                              name=f"oT_{b}_{qc}")
                        cp = nc.scalar.copy if qc % 2 else nc.vector.tensor_copy
                        cp(oTs[qc][:], oTps[qc][:])
                    for qc in range(nqc):
                        on_act = qc in (1, 2)
                        cp = nc.scalar.copy if on_act else nc.vector.tensor_copy
                        out_sb = outpool.tile([P, D], f16, tag="out",
                                              name=f"out_{b}_{qc}")
                        row = row0 + qc * P
                        for half in range(2):
                            # qc0/1's proj accumulators borrow the pvps
                            # banks (free after the divisions) so the tail
                            # projections pipeline over 4 PSUM slots
                            pool_ = pvps if qc < 2 else accp
                            pp = pool_.tile([P, 512], f32,
                                            tag="pv" if qc < 2 else "acc",
                                            name=f"pp_{b}_{qc}_{half}")
                            nc.tensor.matmul(
                                pp[:], oTs[qc][:],
                                wp_sb[:, half * 512:(half + 1) * 512],
                            )
                            dslc = out_sb[:, half * 512:(half + 1) * 512]
                            cp(dslc, pp[:])
                            nc.sync.dma_start(
                                out_d.ap()[row:row + P,
                                           half * 512:(half + 1) * 512],
                                dslc,
                            )

                def project_stage(items, on_act=False):
                    """Breadth-first projection for 1-2 queued qc's: all
                    transposes first, then the oT drains, then per-qc proj
                    matmuls + out drains + DMAs — each proj matmul's oT
                    drain runs during the other qc's transpose, so the PE
                    never head-stalls on a DVE drain."""
                    cp = nc.scalar.copy if on_act else nc.vector.tensor_copy
                    oTps, oTs = [], []
                    for pb, prow0, qc, ov in items:
                        t = accp.tile([P, P], bf16, tag="acc",
                                      name=f"oT_ps_{pb}_{qc}")
                        nc.tensor.transpose(t[:], ov[:], ident[:])
                        oTps.append(t)
                    for i, (pb, prow0, qc, ov) in enumerate(items):
                        t = otpool.tile([P, P], bf16, tag="oT",
                                        name=f"oT_{pb}_{qc}")
                        cp(t[:], oTps[i][:])
                        oTs.append(t)
                    for i, (pb, prow0, qc, ov) in enumerate(items):
                        out_sb = outpool.tile([P, D], f16, tag="out",
                                              name=f"out_{pb}_{qc}")
                        row = prow0 + qc * P
                        for half in range(2):
                            pp = accp.tile([P, 512], f32, tag="acc",
                                           name=f"pp_{pb}_{qc}_{half}")
                            nc.tensor.matmul(
                                pp[:], oTs[i][:],
                                wp_sb[:, half * 512:(half + 1) * 512],
                            )
                            dslc = out_sb[:, half * 512:(half + 1) * 512]
                            cp(dslc, pp[:])
                            nc.sync.dma_start(
                                out_d.ap()[row:row + P,
                                           half * 512:(half + 1) * 512],
                                dslc,
                            )

                def project_qc(b, row0, qc, ov, on_act=False):
                    project_stage([(b, row0, qc, ov)], on_act=on_act)

                # ================= main schedule =================
                assert nb == qw, "block size must match j-slice width"
                blocks = [nb] * n_blocks
                mcs_per_j = qw // P

                # PV/div/proj work queue: entries (b, row0, qc), popped into
                # later blocks' PE slack once all of v_sb has been emitted.
                from collections import deque
                pvq = deque()

                PROBE = os.environ.get("ATTN_PROBE", "0") == "1"
                projq = deque()   # (b, row0, qc, ov) divided, awaiting proj

                def pop_pv(k, last=False):
                    """Pop k PV+division units, then one deferred projection.

                    Projections lag divisions by one pop so the DVE never
                    has a projection drain queued ahead of the division the
                    next PV stream's PSUM-slot reuse is waiting on."""
                    if PROBE:
                        pvq.clear()
                        return
                    for _ in range(k):
                        if not pvq:
                            break
                        pb, prow0, qc = pvq.popleft()
                        ov = pv_div_qc(pb, nb, qc)
                        projq.append((pb, prow0, qc, ov))
                    keep = 0 if last else int(os.environ.get('ATTN_KEEP', '2'))
                    batch = []
                    while len(projq) > keep:
                        batch.append(projq.popleft())
                    if batch:
                        project_stage(batch)

                row0 = 0
                for b, nbb in enumerate(blocks):
                    if b == 0:
                        # interleave with QKV j-sweeps: k(j) gates scores of
                        # its m-chunks and is always the next x-slice off the
                        # wire - nothing else rides ahead of it on the PE;
                        # v(j0)/q(j1) fill the PE after the last k lands.
                        qkv_j0_qk(lambda c: scores_exp(0, 0, nbb, c))
                        for j in range(1, n_j):
                            qkv_j(j, parts=(1,))
                            for mc in range(j * mcs_per_j, (j + 1) * mcs_per_j):
                                scores_exp(b, row0, nbb, mc)
                        # q(j1) first so its drain overlaps the v compute;
                        # v(j1) rides here too (b0 has PE slack while ACT
                        # chews the backlog; b1's slots then take pops)
                        qkv_j(1, parts=(0,))
                        qkv_j(0, parts=(2,))
                        qkv_j(1, parts=(2,))
                    elif b < n_blocks - 1:
                        # per-mc interleave: at most one deferred unit rides
                        # after each scores tile, so PE bursts stay inside
                        # the 2-tile sps elasticity window and ACT never
                        # starves; v(j1..3)/q(j+1) land in their own slots
                        vjs = {2: 2, 5: 3} if b == 1 else {}
                        qj = {8: 2} if b == 1 else {6: 3}
                        # pops only once v_sb is fully emitted (v(j3) @ b1mc5)
                        pops = ({7: 1, 10: 1, 12: 1, 14: 1} if b == 1 else
                                {0: 1, 3: 1, 8: 1, 11: 1, 14: 1})
                        for mc in range(m_chunks):
                            if pops.get(mc) and pvq:
                                # pre-emit the pv pair's memset so the pop's
                                # first matmul never head-stalls the PE on it
                                nb_, _, nqc = pvq[0]
                                if (nb_, nqc // 2) not in pv_tiles:
                                    pv_tiles[(nb_, nqc // 2)] = \
                                        pv_qcpair_psum(nb_, nqc // 2)
                            scores_exp(b, row0, nbb, mc)
                            if mc in vjs:
                                qkv_j(vjs[mc], parts=(2,))
                            if mc in qj and qj[mc] < n_j:
                                qkv_j(qj[mc], parts=(0,))
                            pop_pv(pops.get(mc, 0))
                    else:
                        # last block: mc-major PV rides along with the exp
                        # stream; each qc-pair starts only after the pops
                        # that free its PSUM slot (avoids a DVE-order
                        # deadlock on the memset's WAR)
                        pops = {0: 1, 1: 1, 4: 1}
                        for g in range(4):
                            for mc in range(g * 4, g * 4 + 4):
                                if pops.get(mc) and pvq:
                                    nb_, _, nqc = pvq[0]
                                    if (nb_, nqc // 2) not in pv_tiles:
                                        pv_tiles[(nb_, nqc // 2)] = \
                                            pv_qcpair_psum(nb_, nqc // 2)
                                scores_exp(b, row0, nbb, mc)
                                pop_pv(pops.get(mc, 0))
                            if g == 0:
                                pv_mc_last(b, nbb, 0, range(0, 4))
                            elif g == 1:
                                pv_mc_last(b, nbb, 0, range(4, 8))
                                pv_mc_last(b, nbb, 1, range(0, 8))
                            else:
                                pv_mc_last(b, nbb, 0, range(g * 4, g * 4 + 4))
                                pv_mc_last(b, nbb, 1, range(g * 4, g * 4 + 4))
                    if b < n_blocks - 1:
                        for qc in range(QC):
                            pvq.append((b, row0, qc))
                    row0 += nbb
                # flush any queue leftovers, then the last block's tail
                pop_pv(len(pvq), last=False)
                while projq:
                    batch = [projq.popleft() for _ in range(min(2, len(projq)))]
                    project_stage(batch)
                div_proj_last(n_blocks - 1, (n_blocks - 1) * nb, nb)

            pvps.release()
            sps.release()
            accp.release()

    nc.compile()
    return nc


def _host_prep(x, w_qkv, b_qkv, w_proj, n=N):
    """Per-core input maps (dtypes match the DRAM tensor declarations)."""
    import ml_dtypes

    bf = ml_dtypes.bfloat16
    xT = np.ascontiguousarray(x.T.astype(bf))
    in_maps = []
    for c in range(NCORES):
        wq = w_qkv[0 * D + c * P:0 * D + (c + 1) * P, :]
        wk = w_qkv[1 * D + c * P:1 * D + (c + 1) * P, :]
        wv = w_qkv[2 * D + c * P:2 * D + (c + 1) * P, :]
        # part-major [p, part, o, c]: contiguous per-part weight DMAs
        wqkvT = np.ascontiguousarray(
            np.stack(
                [a.T.reshape(D_CHUNKS, P, P).transpose(1, 0, 2) for a in (wq, wk, wv)],
                axis=1,
            ).astype(bf)
        )
        wpT = np.ascontiguousarray(w_proj[:, c * P:(c + 1) * P].T.astype(bf))
        bq = b_qkv[0 * D + c * P:0 * D + (c + 1) * P]
        bk = b_qkv[1 * D + c * P:1 * D + (c + 1) * P]
        bv = b_qkv[2 * D + c * P:2 * D + (c + 1) * P]
        bqkv = np.ascontiguousarray(
            np.stack([bq, bk, bv], axis=1).astype(np.float32)
        )
        bvrow = np.ascontiguousarray(bv.reshape(1, P).astype(bf))
        in_maps.append({"xT": xT, "wqkvT": wqkvT, "wpT": wpT, "bqkv": bqkv,
                        "bvrow": bvrow})
    return in_maps


_NC_CACHE = {}


def run(x, w_qkv, b_qkv, w_proj, b_proj, trace=False, n=N, nb=None, **spmd_kwargs):
    from concourse.bass_utils import run_bass_kernel_spmd

    if nb is None:
        nb = NB
    key = (n, nb, FP8_JS, DVE_EXP, EXP_B, WARMUP, ES_BUFS,
           os.environ.get("ATTN_POPS", ""), os.environ.get("ATTN_SPLIT_X0", ""),
           os.environ.get("ATTN_KEEP", ""))
    if key not in _NC_CACHE:
        _NC_CACHE[key] = _build_nc(n=n, nb=nb)
    nc = _NC_CACHE[key]

    in_maps = _host_prep(
        np.asarray(x), np.asarray(w_qkv), np.asarray(b_qkv), np.asarray(w_proj), n=n
    )
    results = run_bass_kernel_spmd(
        nc, in_maps, core_ids=list(range(NCORES)), trace=trace, **spmd_kwargs
    )
    acc = np.zeros((n, D), dtype=np.float64)
    for c in range(NCORES):
        acc += results.results[c]["out_part"].astype(np.float64)
    acc += np.asarray(b_proj).astype(np.float64)
    return acc.astype(np.float32), results


def kernel(x, w_qkv, b_qkv, w_proj, b_proj):
    out, _ = run(x, w_qkv, b_qkv, w_proj, b_proj, trace=False)
    return out

